# revision 22
# baseline (speedup 1.0000x reference)
"""Trainium2 Bass kernel for nn_Cifar10_JointMembership.

Math (closed form of the reference 2-qubit circuit; verified vs reference):
  a = x[b, i0], b_ = x[b, i1]  (gathered pixel pairs, full angles)
  out[b, 2p,   c] = 0.5 + 0.5*cos(theta_c)*cos(a) - 0.5*sin(theta_c)*sin(a)*sin(b_)
  out[b, 2p+1, c] = 0.5 + 0.5*cos(a)*cos(b_)               (same for all c)

Sharding: pure data parallel, batch dim split across 8 NeuronCores
(128 rows per core); theta replicated. Full inputs in, full output out.

Per-core pipeline:
  DMA x [128,3072] f32 and pair_idx [128,920] i32 -> SBUF
  u16 index extraction (bitcast + stride-2 copy)
  GPSIMD indirect_copy gather: each Q7 core's 16-partition group uses its
    own wrapped index list; output column i = 16*m + w holds row-w-of-group's
    m-th gathered value looked up in every partition of the group, so the
    valid value for partition p sits at column 16*m + (p % 16).
  Phase merge on the (otherwise idle) TensorEngine: 16 accumulated matmuls
    with diagonal 0/1 weights W_w = diag(p%16 == w) select the valid phase
    per partition into PSUM: packed[p,m] = sum_w W_w[p,p] * tmp[p, 16m+w].
    Exact in fp32 (one nonzero term per output).
  add_range_wrap (custom DVE) to bring angles into [-pi, pi], ACT Sin for
    cos/sin; DVE products; per-class affine (ACT Copy with per-partition
    scale + DVE scalar_tensor_tensor); broadcast odd columns; DMA out.
"""

import os

os.environ.setdefault("BY_DEFAULT_DISABLE_SUBTILE_DEPS", "1")

import numpy as np

import concourse.bass as bass
import concourse.mybir as mybir
from concourse.tile import TileContext as _TileContext, ScopedClock

N_CORES = 8
B_FULL = 1024
B = B_FULL // N_CORES  # 128 rows per core
NPIX = 3072
NPAIR = 460
NIDX = 2 * NPAIR  # 920 gathered values per row
NCLS = 10
F32 = mybir.dt.float32
I32 = mybir.dt.int32
U16 = mybir.dt.uint16
ALU = mybir.AluOpType
PI = float(np.pi)
TWO_PI = float(2 * np.pi)
HALF_PI = float(np.pi / 2)


class TileContext(_TileContext):
    pass


def _legalize_sync_waits(nc):
    """This walrus build allows only ONE sync-wait per non-EventSemaphore
    instruction (and two on EventSemaphore). Tile's add_semaphores can attach
    several. Hoist excess waits onto EventSemaphore instructions inserted
    immediately before the owner on the same engine — semantically identical
    (same engine stream, waits run first)."""
    n_new = 0
    for f in nc.m.functions:
        for bb in f.blocks:
            out = []
            for inst in bb.instructions:
                si = inst.sync_info
                waits = list(si.on_wait) if si is not None and si.on_wait else []
                cap = 2 if inst.opcode == "EventSemaphore" else 1
                if len(waits) > cap:
                    keep, hoist = waits[:cap], waits[cap:]
                    del si.on_wait[:]
                    for w in keep:
                        si.on_wait.append(w)
                    while hoist:
                        chunk, hoist = hoist[:2], hoist[2:]
                        n_new += 1
                        ev = mybir.InstEventSemaphore(
                            name=f"{inst.name}-hw{n_new}",
                            ins=[],
                            outs=[],
                            engine=inst.engine,
                            sync_info=mybir.SyncInfo(on_wait=chunk, on_update=[]),
                        )
                        out.append(ev)
                out.append(inst)
            bb.instructions = out
    return nc


def build_kernel(n_chunks=4, n_repeat=1, pe_phases=12, span_chunks=(2, 1, 1)):
    """One NeuronCore's program: 128 batch rows.

    n_chunks: gather/merge/trig pipeline granularity (divides 920, even CH).
    pe_phases: how many of the 16 phase-merge terms run on the TensorEngine
      (accumulated in PSUM); the rest run as a masked mul-add chain on DVE.
    span_chunks: class/output-stage spans, in units of chunks (sums to
      n_chunks). A small final span shortens the non-overlapped tail.
    n_repeat: re-runs the whole pipeline (identical results) for timing.
    """
    Sin = mybir.ActivationFunctionType.Sin
    Copy = mybir.ActivationFunctionType.Copy
    Abs = mybir.ActivationFunctionType.Abs

    nc = bass.Bass(detect_race_conditions=False)
    xd = nc.dram_tensor("x", [B, NPIX], F32, kind="ExternalInput")
    pd = nc.dram_tensor("pidx", [B, NIDX], I32, kind="ExternalInput")
    td = nc.dram_tensor("theta", [1, NCLS], F32, kind="ExternalInput")
    od = nc.dram_tensor("out", [B, NIDX * NCLS], F32, kind="ExternalOutput")

    assert NIDX % n_chunks == 0
    CH = NIDX // n_chunks  # gathered values per chunk
    assert CH % 2 == 0
    PCH = CH // 2  # pairs per chunk
    assert sum(span_chunks) == n_chunks

    with TileContext(nc) as tc:
        with (
            tc.tile_pool(name="const", bufs=1) as cpool,
            tc.tile_pool(name="inp", bufs=1) as ipool,
            tc.tile_pool(name="tmp", bufs=3) as tpool,
            tc.tile_pool(name="mid", bufs=2) as mpool,
            tc.tile_pool(name="trig", bufs=1) as gpool,
            tc.tile_pool(name="outp", bufs=2) as opool,
            tc.tile_pool(name="tccp", bufs=4) as tccpool,
            tc.tile_pool(name="ps", bufs=2, space="PSUM") as ppool,
        ):
            # --- phase masks M[p, w] = 1.0 if p % 16 == w else 0.0 ---
            rowx = cpool.tile([B, 1], I32, tag="rowx")
            nc.gpsimd.iota(rowx[:], pattern=[[0, 1]], base=0, channel_multiplier=1)
            pm16 = cpool.tile([B, 1], I32, tag="pm16")
            nc.vector.tensor_scalar(pm16[:], rowx[:], 15, None, ALU.bitwise_and)
            wrow = cpool.tile([B, 16], I32, tag="wrow")
            nc.gpsimd.iota(wrow[:], pattern=[[1, 16]], base=0, channel_multiplier=0)
            M = cpool.tile([B, 16], F32, tag="M")
            nc.vector.tensor_tensor(
                M[:], pm16[:, 0:1].broadcast_to((B, 16)), wrow[:], ALU.is_equal
            )

            # --- coefficients: A = 0.5*cos(theta), Bc = -0.5*sin(theta) ---
            # (theta + indices ride the ACT HWDGE ring so the big x DMA on
            # the SP ring doesn't serialize ahead of them)
            th = cpool.tile([B, NCLS], F32, tag="th")
            nc.scalar.dma_start(out=th[:], in_=td[:].to_broadcast((B, NCLS)))
            halfpi = cpool.tile([B, 1], F32, tag="halfpi")
            nc.gpsimd.memset(halfpi[:], HALF_PI)
            zbias = cpool.tile([B, 1], F32, tag="zbias")
            nc.gpsimd.memset(zbias[:], 0.0)

            # Range reduction with standard ALUs (valid for |x| < 3pi):
            #   y = x - 2pi*(x > pi) + 2pi*(x < -pi)  in [-pi, pi]
            #   sin(x) = Sin(y);  cos(x) = cos(|y|) = Sin(-|y| + pi/2)
            def wrap2(dst_y, src, g, l, y1, cmp_engine):
                cmp_engine.tensor_scalar(g, src, PI, None, ALU.is_gt)
                cmp_engine.tensor_scalar(l, src, -PI, None, ALU.is_lt)
                nc.vector.scalar_tensor_tensor(
                    y1, g, -TWO_PI, src, ALU.mult, ALU.add
                )
                nc.vector.scalar_tensor_tensor(
                    dst_y, l, TWO_PI, y1, ALU.mult, ALU.add
                )

            thy = cpool.tile([B, NCLS], F32, tag="thy")
            thz = cpool.tile([B, NCLS], F32, tag="thz")
            tg = cpool.tile([B, NCLS], F32, tag="tg")
            tl = cpool.tile([B, NCLS], F32, tag="tl")
            t1 = cpool.tile([B, NCLS], F32, tag="t1")
            wrap2(thy[:], th[:], tg[:], tl[:], t1[:], nc.vector)
            nc.scalar.activation(thz[:], thy[:], Abs, bias=zbias[:, 0:1])
            A = cpool.tile([B, NCLS], F32, tag="A")
            Bc = cpool.tile([B, NCLS], F32, tag="Bc")
            nc.scalar.activation(A[:], thz[:], Sin, bias=halfpi[:, 0:1], scale=-1.0)
            nc.scalar.activation(Bc[:], thy[:], Sin, bias=zbias[:, 0:1])
            nc.vector.tensor_scalar_mul(A[:], A[:], 0.5)
            nc.vector.tensor_scalar_mul(Bc[:], Bc[:], -0.5)

            if pe_phases > 0:
                # --- diagonal selection weights W_w = diag(M[:, w]) ---
                colx = cpool.tile([B, 128], I32, tag="colx")
                nc.gpsimd.iota(colx[:], pattern=[[1, 128]], base=0, channel_multiplier=0)
                D = cpool.tile([B, 128], F32, tag="D")
                nc.vector.tensor_tensor(
                    D[:], rowx[:, 0:1].broadcast_to((B, 128)), colx[:], ALU.is_equal
                )
                W_all = cpool.tile([B, pe_phases * 128], F32, tag="W_all")
                for w in range(pe_phases):
                    nc.vector.tensor_scalar(
                        W_all[:, w * 128 : (w + 1) * 128],
                        D[:],
                        M[:, w : w + 1],
                        None,
                        ALU.mult,
                    )

            # --- inputs ---
            i32 = ipool.tile([B, NIDX], I32, tag="i32")
            nc.scalar.dma_start(out=i32[:], in_=pd[:])
            xt = ipool.tile([B, NPIX], F32, tag="xt")
            nc.sync.dma_start(out=xt[:], in_=xd[:])
            i16 = ipool.tile([B, NIDX], U16, tag="i16")
            nc.vector.tensor_copy(i16[:], i32[:].bitcast(U16)[:, 0 : 2 * NIDX : 2])

            if pe_phases > 0:
                # warm the PE (HAM) during the prologue so the first real
                # merge matmuls run at full rate
                pwarm = ppool.tile([B, 64], F32, tag="pwarm")
                for _ in range(6):
                    nc.tensor.matmul(
                        pwarm[:], W_all[:, 0:128], D[:, 0:64], start=True, stop=True
                    )

            def class_span(p0, PS, ca, sa, cb, sb):
                """Per-class output stage for pairs [p0, p0+PS)."""
                sl = slice(p0, p0 + PS)
                v = mpool.tile([B, PS], F32, tag="v")
                wv = mpool.tile([B, PS], F32, tag="wv")
                nc.vector.tensor_mul(v[:], sa[:, sl], sb[:, sl])
                nc.vector.tensor_mul(wv[:], ca[:, sl], cb[:, sl])

                ob = opool.tile([B, PS * 2 * NCLS], F32, tag="ob")
                ob3 = ob[:].rearrange("p (t k) -> p t k", k=2 * NCLS)

                om = mpool.tile([B, PS], F32, tag="om")
                nc.scalar.activation(om[:], wv[:], Copy, bias=0.5, scale=0.5)
                nc.scalar.activation(
                    ob3[:, :, NCLS : 2 * NCLS],
                    om[:, :, None].broadcast_to((B, PS, NCLS)),
                    Copy,
                )

                # even rows per class c: A_c*ca + (Bc_c*v + 0.5)
                for c in range(NCLS):
                    tcc = tccpool.tile([B, PS], F32, tag="tcc")
                    nc.scalar.activation(
                        tcc[:], v[:], Copy, bias=0.5, scale=Bc[:, c : c + 1]
                    )
                    nc.vector.scalar_tensor_tensor(
                        ob[:, c : PS * 2 * NCLS : 2 * NCLS],
                        ca[:, sl],
                        A[:, c : c + 1],
                        tcc[:],
                        ALU.mult,
                        ALU.add,
                    )

                nc.sync.dma_start(
                    out=od[:, p0 * 2 * NCLS : (p0 + PS) * 2 * NCLS],
                    in_=ob[:],
                )

            for rep in range(n_repeat):
                # full-width cos/sin buffers, filled per chunk
                ca = gpool.tile([B, NPAIR], F32, tag="ca")
                sa = gpool.tile([B, NPAIR], F32, tag="sa")
                cb = gpool.tile([B, NPAIR], F32, tag="cb")
                sb = gpool.tile([B, NPAIR], F32, tag="sb")

                for k in range(n_chunks):
                    # --- gather (this walrus caps indirect_copy at 64
                    # index-columns per instruction, so sub-gather) ---
                    tmp = tpool.tile([B, CH * 16], F32, tag="tmp")
                    GSUB = 64
                    for g0 in range(0, CH, GSUB):
                        gn = min(GSUB, CH - g0)
                        nc.gpsimd.indirect_copy(
                            tmp[:, 16 * g0 : 16 * (g0 + gn)],
                            xt[:],
                            i16[:, k * CH + g0 : k * CH + g0 + gn],
                            True,
                        )

                    # --- phase merge -> pk[p, m] (interleaved a,b) ---
                    if pe_phases > 0:
                        pk = ppool.tile([B, CH], F32, tag="pk")
                        for w in range(pe_phases):
                            nc.tensor.matmul(
                                pk[:],
                                W_all[:, w * 128 : (w + 1) * 128],
                                tmp[:, w : CH * 16 : 16],
                                start=(w == 0),
                                stop=(w == pe_phases - 1),
                            )
                    if pe_phases < 16:
                        w0 = pe_phases
                        acc = mpool.tile([B, CH], F32, tag="acc")
                        nc.vector.tensor_scalar(
                            acc[:], tmp[:, w0 : CH * 16 : 16], M[:, w0 : w0 + 1],
                            None, ALU.mult,
                        )
                        for w in range(w0 + 1, 16):
                            nc.vector.scalar_tensor_tensor(
                                acc[:],
                                tmp[:, w : CH * 16 : 16],
                                M[:, w : w + 1],
                                acc[:],
                                ALU.mult,
                                ALU.add,
                            )
                        if pe_phases > 0:
                            nc.vector.tensor_add(acc[:], acc[:], pk[:])
                        pk = acc

                    # --- range reduction + trig -> full-width buffers ---
                    # (pk is SBUF when the DVE-assist merge ran; evacuate
                    # PSUM via ACT first when the full merge was on PE, so
                    # the GPSIMD compares can read it)
                    if pe_phases == 16:
                        pksb = mpool.tile([B, CH], F32, tag="pksb")
                        nc.scalar.activation(pksb[:], pk[:], Copy)
                        pk = pksb
                    av = pk[:, 0:CH:2]
                    bv = pk[:, 1:CH:2]
                    aw = mpool.tile([B, PCH], F32, tag="aw")
                    ac = mpool.tile([B, PCH], F32, tag="ac")
                    bw = mpool.tile([B, PCH], F32, tag="bw")
                    bc2 = mpool.tile([B, PCH], F32, tag="bc2")
                    ga = mpool.tile([B, PCH], F32, tag="ga")
                    la = mpool.tile([B, PCH], F32, tag="la")
                    gb = mpool.tile([B, PCH], F32, tag="gb")
                    lb = mpool.tile([B, PCH], F32, tag="lb")
                    wrap2(aw[:], av, ga[:], la[:], ac[:], nc.gpsimd)
                    nc.scalar.activation(ac[:], aw[:], Abs, bias=zbias[:, 0:1])
                    wrap2(bw[:], bv, gb[:], lb[:], bc2[:], nc.gpsimd)
                    nc.scalar.activation(bc2[:], bw[:], Abs, bias=zbias[:, 0:1])

                    sl = slice(k * PCH, (k + 1) * PCH)
                    nc.scalar.activation(
                        ca[:, sl], ac[:], Sin, bias=halfpi[:, 0:1], scale=-1.0
                    )
                    nc.scalar.activation(sa[:, sl], aw[:], Sin, bias=zbias[:, 0:1])
                    nc.scalar.activation(
                        cb[:, sl], bc2[:], Sin, bias=halfpi[:, 0:1], scale=-1.0
                    )
                    nc.scalar.activation(sb[:, sl], bw[:], Sin, bias=zbias[:, 0:1])

                    # interleave the class/output stage as soon as its
                    # span of pairs is complete
                    done = k + 1
                    acc_ch = 0
                    for nch in span_chunks:
                        if acc_ch + nch == done:
                            class_span(acc_ch * PCH, nch * PCH, ca, sa, cb, sb)
                            break
                        acc_ch += nch
    return _legalize_sync_waits(nc)


def _prep_inputs(x, theta, pair_idx):
    """Full inputs -> list of per-core input maps (host-side sharding only)."""
    x = np.ascontiguousarray(np.asarray(x, dtype=np.float32).reshape(B_FULL, NPIX))
    theta = np.ascontiguousarray(np.asarray(theta, dtype=np.float32).reshape(1, NCLS))
    pidx = np.asarray(pair_idx)
    assert pidx.shape == (B_FULL, NPAIR, 2), pidx.shape
    if pidx.dtype != np.int32:
        pidx = pidx.astype(np.int32)  # value-preserving narrowing for the DMA
    pidx = np.ascontiguousarray(pidx.reshape(B_FULL, NIDX))
    in_maps = []
    for k in range(N_CORES):
        sl = slice(k * B, (k + 1) * B)
        in_maps.append({"x": x[sl], "pidx": pidx[sl], "theta": theta})
    return in_maps


_CACHED = {}


def kernel(x, theta, pair_idx):
    from concourse.bass_utils import run_bass_kernel_spmd

    if "nc" not in _CACHED:
        _CACHED["nc"] = build_kernel()
    nc = _CACHED["nc"]
    in_maps = _prep_inputs(x, theta, pair_idx)
    res = run_bass_kernel_spmd(nc, in_maps, core_ids=list(range(N_CORES)))
    out = np.concatenate([r["out"] for r in res.results], axis=0)
    return out.reshape(B_FULL, NIDX, NCLS)


# revision 24
# speedup vs baseline: 8.3694x; 8.3694x over previous
"""Trainium2 Bass kernel for nn_Cifar10_JointMembership.

Math (closed form of the reference 2-qubit circuit; verified vs reference):
  a = x[b, i0], b_ = x[b, i1]  (gathered pixel pairs, full angles)
  out[b, 2p,   c] = 0.5 + 0.5*cos(theta_c)*cos(a) - 0.5*sin(theta_c)*sin(a)*sin(b_)
  out[b, 2p+1, c] = 0.5 + 0.5*cos(a)*cos(b_)               (same for all c)

Sharding: pure data parallel, batch dim split across 8 NeuronCores
(128 rows per core); theta replicated. Full inputs in, full output out.

Per-core pipeline:
  DMA x [128,3072] f32 and pair_idx [128,920] i32 -> SBUF
  u16 index extraction (bitcast + stride-2 copy)
  GPSIMD indirect_copy gather: each Q7 core's 16-partition group uses its
    own wrapped index list; output column i = 16*m + w holds row-w-of-group's
    m-th gathered value looked up in every partition of the group, so the
    valid value for partition p sits at column 16*m + (p % 16).
  Phase merge on the (otherwise idle) TensorEngine: 16 accumulated matmuls
    with diagonal 0/1 weights W_w = diag(p%16 == w) select the valid phase
    per partition into PSUM: packed[p,m] = sum_w W_w[p,p] * tmp[p, 16m+w].
    Exact in fp32 (one nonzero term per output).
  Range reduction with compare-wraps (is_gt/is_lt + mul-add) into
    [-pi, pi]; ACT Sin for sin, Sin(-|y|+pi/2) for cos; DVE products;
    per-class affine (ACT Copy with per-partition scale + DVE
    scalar_tensor_tensor); broadcast odd columns; DMA out.
"""

import os

os.environ.setdefault("BY_DEFAULT_DISABLE_SUBTILE_DEPS", "1")

import numpy as np

import concourse.bass as bass
import concourse.mybir as mybir
from concourse.tile import TileContext as _TileContext, ScopedClock

N_CORES = 8
B_FULL = 1024
B = B_FULL // N_CORES  # 128 rows per core
NPIX = 3072
NPAIR = 460
NIDX = 2 * NPAIR  # 920 gathered values per row
NCLS = 10
F32 = mybir.dt.float32
I32 = mybir.dt.int32
U16 = mybir.dt.uint16
ALU = mybir.AluOpType
PI = float(np.pi)
TWO_PI = float(2 * np.pi)
HALF_PI = float(np.pi / 2)


class TileContext(_TileContext):
    pass


def _legalize_sync_waits(nc):
    """This walrus build allows only ONE sync-wait per non-EventSemaphore
    instruction (and two on EventSemaphore). Tile's add_semaphores can attach
    several. Hoist excess waits onto EventSemaphore instructions inserted
    immediately before the owner on the same engine — semantically identical
    (same engine stream, waits run first)."""
    n_new = 0
    for f in nc.m.functions:
        for bb in f.blocks:
            out = []
            for inst in bb.instructions:
                si = inst.sync_info
                waits = list(si.on_wait) if si is not None and si.on_wait else []
                cap = 2 if inst.opcode == "EventSemaphore" else 1
                if len(waits) > cap:
                    keep, hoist = waits[:cap], waits[cap:]
                    del si.on_wait[:]
                    for w in keep:
                        si.on_wait.append(w)
                    while hoist:
                        chunk, hoist = hoist[:2], hoist[2:]
                        n_new += 1
                        ev = mybir.InstEventSemaphore(
                            name=f"{inst.name}-hw{n_new}",
                            ins=[],
                            outs=[],
                            engine=inst.engine,
                            sync_info=mybir.SyncInfo(on_wait=chunk, on_update=[]),
                        )
                        out.append(ev)
                out.append(inst)
            bb.instructions = out
    return nc


def build_kernel(n_chunks=4, n_repeat=1, pe_phases=12, span_chunks=(2, 1, 1), parts="gmtc"):
    """One NeuronCore's program: 128 batch rows.

    n_chunks: gather/merge/trig pipeline granularity (divides 920, even CH).
    pe_phases: how many of the 16 phase-merge terms run on the TensorEngine
      (accumulated in PSUM); the rest run as a masked mul-add chain on DVE.
    span_chunks: class/output-stage spans, in units of chunks (sums to
      n_chunks). A small final span shortens the non-overlapped tail.
    n_repeat: re-runs the whole pipeline (identical results) for timing.
    """
    Sin = mybir.ActivationFunctionType.Sin
    Copy = mybir.ActivationFunctionType.Copy
    Abs = mybir.ActivationFunctionType.Abs

    nc = bass.Bass(detect_race_conditions=False)
    xd = nc.dram_tensor("x", [B, NPIX], F32, kind="ExternalInput")
    pd = nc.dram_tensor("pidx", [B, NIDX], I32, kind="ExternalInput")
    td = nc.dram_tensor("theta", [1, NCLS], F32, kind="ExternalInput")
    od = nc.dram_tensor("out", [B, NIDX * NCLS], F32, kind="ExternalOutput")

    assert NIDX % n_chunks == 0
    CH = NIDX // n_chunks  # gathered values per chunk
    assert CH % 2 == 0
    PCH = CH // 2  # pairs per chunk
    assert sum(span_chunks) == n_chunks

    with TileContext(nc) as tc:
        with (
            tc.tile_pool(name="const", bufs=1) as cpool,
            tc.tile_pool(name="inp", bufs=1) as ipool,
            tc.tile_pool(name="tmp", bufs=3) as tpool,
            tc.tile_pool(name="mid", bufs=2) as mpool,
            tc.tile_pool(name="trig", bufs=1) as gpool,
            tc.tile_pool(name="outp", bufs=2) as opool,
            tc.tile_pool(name="tccp", bufs=4) as tccpool,
            tc.tile_pool(name="ps", bufs=2, space="PSUM") as ppool,
        ):
            # --- phase masks M[p, w] = 1.0 if p % 16 == w else 0.0 ---
            rowx = cpool.tile([B, 1], I32, tag="rowx")
            nc.gpsimd.iota(rowx[:], pattern=[[0, 1]], base=0, channel_multiplier=1)
            pm16 = cpool.tile([B, 1], I32, tag="pm16")
            nc.vector.tensor_scalar(pm16[:], rowx[:], 15, None, ALU.bitwise_and)
            wrow = cpool.tile([B, 16], I32, tag="wrow")
            nc.gpsimd.iota(wrow[:], pattern=[[1, 16]], base=0, channel_multiplier=0)
            M = cpool.tile([B, 16], F32, tag="M")
            nc.vector.tensor_tensor(
                M[:], pm16[:, 0:1].broadcast_to((B, 16)), wrow[:], ALU.is_equal
            )

            # --- coefficients: A = 0.5*cos(theta), Bc = -0.5*sin(theta) ---
            # (theta + indices ride the ACT HWDGE ring so the big x DMA on
            # the SP ring doesn't serialize ahead of them)
            th = cpool.tile([B, NCLS], F32, tag="th")
            nc.scalar.dma_start(out=th[:], in_=td[:].to_broadcast((B, NCLS)))
            halfpi = cpool.tile([B, 1], F32, tag="halfpi")
            nc.gpsimd.memset(halfpi[:], HALF_PI)
            zbias = cpool.tile([B, 1], F32, tag="zbias")
            nc.gpsimd.memset(zbias[:], 0.0)

            # Range reduction with standard ALUs (valid for |x| < 3pi):
            #   y = x - 2pi*(x > pi) + 2pi*(x < -pi)  in [-pi, pi]
            #   sin(x) = Sin(y);  cos(x) = cos(|y|) = Sin(-|y| + pi/2)
            def wrap2(dst_y, src, g, l, y1, cmp_engine):
                cmp_engine.tensor_scalar(g, src, PI, None, ALU.is_gt)
                cmp_engine.tensor_scalar(l, src, -PI, None, ALU.is_lt)
                nc.vector.scalar_tensor_tensor(
                    y1, g, -TWO_PI, src, ALU.mult, ALU.add
                )
                nc.vector.scalar_tensor_tensor(
                    dst_y, l, TWO_PI, y1, ALU.mult, ALU.add
                )

            thy = cpool.tile([B, NCLS], F32, tag="thy")
            thz = cpool.tile([B, NCLS], F32, tag="thz")
            tg = cpool.tile([B, NCLS], F32, tag="tg")
            tl = cpool.tile([B, NCLS], F32, tag="tl")
            t1 = cpool.tile([B, NCLS], F32, tag="t1")
            wrap2(thy[:], th[:], tg[:], tl[:], t1[:], nc.vector)
            nc.scalar.activation(thz[:], thy[:], Abs, bias=zbias[:, 0:1])
            A = cpool.tile([B, NCLS], F32, tag="A")
            Bc = cpool.tile([B, NCLS], F32, tag="Bc")
            nc.scalar.activation(A[:], thz[:], Sin, bias=halfpi[:, 0:1], scale=-1.0)
            nc.scalar.activation(Bc[:], thy[:], Sin, bias=zbias[:, 0:1])
            nc.vector.tensor_scalar_mul(A[:], A[:], 0.5)
            nc.vector.tensor_scalar_mul(Bc[:], Bc[:], -0.5)

            if pe_phases > 0:
                # --- diagonal selection weights W_w = diag(M[:, w]) ---
                colx = cpool.tile([B, 128], I32, tag="colx")
                nc.gpsimd.iota(colx[:], pattern=[[1, 128]], base=0, channel_multiplier=0)
                D = cpool.tile([B, 128], F32, tag="D")
                nc.vector.tensor_tensor(
                    D[:], rowx[:, 0:1].broadcast_to((B, 128)), colx[:], ALU.is_equal
                )
                W_all = cpool.tile([B, pe_phases * 128], F32, tag="W_all")
                for w in range(pe_phases):
                    nc.vector.tensor_scalar(
                        W_all[:, w * 128 : (w + 1) * 128],
                        D[:],
                        M[:, w : w + 1],
                        None,
                        ALU.mult,
                    )

            # --- inputs ---
            i32 = ipool.tile([B, NIDX], I32, tag="i32")
            nc.scalar.dma_start(out=i32[:], in_=pd[:])
            xt = ipool.tile([B, NPIX], F32, tag="xt")
            nc.sync.dma_start(out=xt[:], in_=xd[:])
            i16 = ipool.tile([B, NIDX], U16, tag="i16")
            nc.vector.tensor_copy(i16[:], i32[:].bitcast(U16)[:, 0 : 2 * NIDX : 2])

            if pe_phases > 0:
                # warm the PE (HAM) during the prologue so the first real
                # merge matmuls run at full rate
                pwarm = ppool.tile([B, 64], F32, tag="pwarm")
                for _ in range(6):
                    nc.tensor.matmul(
                        pwarm[:], W_all[:, 0:128], D[:, 0:64], start=True, stop=True
                    )

            def class_span(p0, PS, ca, sa, cb, sb):
                """Per-class output stage for pairs [p0, p0+PS)."""
                sl = slice(p0, p0 + PS)
                v = mpool.tile([B, PS], F32, tag="v")
                wv = mpool.tile([B, PS], F32, tag="wv")
                nc.vector.tensor_mul(v[:], sa[:, sl], sb[:, sl])
                nc.vector.tensor_mul(wv[:], ca[:, sl], cb[:, sl])

                ob = opool.tile([B, PS * 2 * NCLS], F32, tag="ob")
                ob3 = ob[:].rearrange("p (t k) -> p t k", k=2 * NCLS)

                om = mpool.tile([B, PS], F32, tag="om")
                nc.scalar.activation(om[:], wv[:], Copy, bias=0.5, scale=0.5)
                nc.scalar.activation(
                    ob3[:, :, NCLS : 2 * NCLS],
                    om[:, :, None].broadcast_to((B, PS, NCLS)),
                    Copy,
                )

                # even rows per class c: A_c*ca + (Bc_c*v + 0.5)
                for c in range(NCLS):
                    tcc = tccpool.tile([B, PS], F32, tag="tcc")
                    nc.scalar.activation(
                        tcc[:], v[:], Copy, bias=0.5, scale=Bc[:, c : c + 1]
                    )
                    nc.vector.scalar_tensor_tensor(
                        ob[:, c : PS * 2 * NCLS : 2 * NCLS],
                        ca[:, sl],
                        A[:, c : c + 1],
                        tcc[:],
                        ALU.mult,
                        ALU.add,
                    )

                nc.sync.dma_start(
                    out=od[:, p0 * 2 * NCLS : (p0 + PS) * 2 * NCLS],
                    in_=ob[:],
                )

            for rep in range(n_repeat):
                # full-width cos/sin buffers, filled per chunk
                ca = gpool.tile([B, NPAIR], F32, tag="ca")
                sa = gpool.tile([B, NPAIR], F32, tag="sa")
                cb = gpool.tile([B, NPAIR], F32, tag="cb")
                sb = gpool.tile([B, NPAIR], F32, tag="sb")

                for k in range(n_chunks):
                    # --- gather (this walrus caps indirect_copy at 64
                    # index-columns per instruction, so sub-gather) ---
                    tmp = tpool.tile([B, CH * 16], F32, tag="tmp")
                    GSUB = 64
                    for g0 in [] if "g" not in parts else range(0, CH, GSUB):
                        gn = min(GSUB, CH - g0)
                        nc.gpsimd.indirect_copy(
                            tmp[:, 16 * g0 : 16 * (g0 + gn)],
                            xt[:],
                            i16[:, k * CH + g0 : k * CH + g0 + gn],
                            True,
                        )

                    # --- phase merge -> pk[p, m] (interleaved a,b) ---
                    if "m" not in parts:
                        pk = mpool.tile([B, CH], F32, tag="acc")
                    elif pe_phases > 0:
                        pk = ppool.tile([B, CH], F32, tag="pk")
                        for w in range(pe_phases):
                            nc.tensor.matmul(
                                pk[:],
                                W_all[:, w * 128 : (w + 1) * 128],
                                tmp[:, w : CH * 16 : 16],
                                start=(w == 0),
                                stop=(w == pe_phases - 1),
                            )
                    if "m" in parts and pe_phases < 16:
                        w0 = pe_phases
                        acc = mpool.tile([B, CH], F32, tag="acc")
                        nc.vector.tensor_scalar(
                            acc[:], tmp[:, w0 : CH * 16 : 16], M[:, w0 : w0 + 1],
                            None, ALU.mult,
                        )
                        for w in range(w0 + 1, 16):
                            nc.vector.scalar_tensor_tensor(
                                acc[:],
                                tmp[:, w : CH * 16 : 16],
                                M[:, w : w + 1],
                                acc[:],
                                ALU.mult,
                                ALU.add,
                            )
                        if pe_phases > 0:
                            nc.vector.tensor_add(acc[:], acc[:], pk[:])
                        pk = acc

                    # --- range reduction + trig -> full-width buffers ---
                    # (pk is SBUF when the DVE-assist merge ran; evacuate
                    # PSUM via ACT first when the full merge was on PE, so
                    # the GPSIMD compares can read it)
                    if pe_phases == 16:
                        pksb = mpool.tile([B, CH], F32, tag="pksb")
                        nc.scalar.activation(pksb[:], pk[:], Copy)
                        pk = pksb
                    av = pk[:, 0:CH:2]
                    bv = pk[:, 1:CH:2]
                    aw = mpool.tile([B, PCH], F32, tag="aw")
                    ac = mpool.tile([B, PCH], F32, tag="ac")
                    bw = mpool.tile([B, PCH], F32, tag="bw")
                    bc2 = mpool.tile([B, PCH], F32, tag="bc2")
                    ga = mpool.tile([B, PCH], F32, tag="ga")
                    la = mpool.tile([B, PCH], F32, tag="la")
                    gb = mpool.tile([B, PCH], F32, tag="gb")
                    lb = mpool.tile([B, PCH], F32, tag="lb")
                    if "t" not in parts:
                        continue
                    wrap2(aw[:], av, ga[:], la[:], ac[:], nc.gpsimd)
                    nc.scalar.activation(ac[:], aw[:], Abs, bias=zbias[:, 0:1])
                    wrap2(bw[:], bv, gb[:], lb[:], bc2[:], nc.gpsimd)
                    nc.scalar.activation(bc2[:], bw[:], Abs, bias=zbias[:, 0:1])

                    sl = slice(k * PCH, (k + 1) * PCH)
                    nc.scalar.activation(
                        ca[:, sl], ac[:], Sin, bias=halfpi[:, 0:1], scale=-1.0
                    )
                    nc.scalar.activation(sa[:, sl], aw[:], Sin, bias=zbias[:, 0:1])
                    nc.scalar.activation(
                        cb[:, sl], bc2[:], Sin, bias=halfpi[:, 0:1], scale=-1.0
                    )
                    nc.scalar.activation(sb[:, sl], bw[:], Sin, bias=zbias[:, 0:1])

                    # interleave the class/output stage as soon as its
                    # span of pairs is complete
                    done = k + 1
                    acc_ch = 0
                    for nch in [] if "c" not in parts else span_chunks:
                        if acc_ch + nch == done:
                            class_span(acc_ch * PCH, nch * PCH, ca, sa, cb, sb)
                            break
                        acc_ch += nch
    return _legalize_sync_waits(nc)


def _prep_inputs(x, theta, pair_idx):
    """Full inputs -> list of per-core input maps (host-side sharding only)."""
    x = np.ascontiguousarray(np.asarray(x, dtype=np.float32).reshape(B_FULL, NPIX))
    theta = np.ascontiguousarray(np.asarray(theta, dtype=np.float32).reshape(1, NCLS))
    pidx = np.asarray(pair_idx)
    assert pidx.shape == (B_FULL, NPAIR, 2), pidx.shape
    if pidx.dtype != np.int32:
        pidx = pidx.astype(np.int32)  # value-preserving narrowing for the DMA
    pidx = np.ascontiguousarray(pidx.reshape(B_FULL, NIDX))
    in_maps = []
    for k in range(N_CORES):
        sl = slice(k * B, (k + 1) * B)
        in_maps.append({"x": x[sl], "pidx": pidx[sl], "theta": theta})
    return in_maps


_CACHED = {}


def kernel(x, theta, pair_idx):
    from concourse.bass_utils import run_bass_kernel_spmd

    if "nc" not in _CACHED:
        _CACHED["nc"] = build_kernel()
    nc = _CACHED["nc"]
    in_maps = _prep_inputs(x, theta, pair_idx)
    res = run_bass_kernel_spmd(nc, in_maps, core_ids=list(range(N_CORES)))
    out = np.concatenate([r["out"] for r in res.results], axis=0)
    return out.reshape(B_FULL, NIDX, NCLS)


# revision 25
# speedup vs baseline: 8.7902x; 1.0503x over previous
"""Trainium2 Bass kernel for nn_Cifar10_JointMembership.

Math (closed form of the reference 2-qubit circuit; verified vs reference):
  a = x[b, i0], b_ = x[b, i1]  (gathered pixel pairs, full angles)
  out[b, 2p,   c] = 0.5 + 0.5*cos(theta_c)*cos(a) - 0.5*sin(theta_c)*sin(a)*sin(b_)
  out[b, 2p+1, c] = 0.5 + 0.5*cos(a)*cos(b_)               (same for all c)

Sharding: pure data parallel, batch dim split across 8 NeuronCores
(128 rows per core); theta replicated. Full inputs in, full output out.

Per-core pipeline:
  DMA x [128,3072] f32 and pair_idx [128,920] i32 -> SBUF
  u16 index extraction (bitcast + stride-2 copy)
  GPSIMD indirect_copy gather: each Q7 core's 16-partition group uses its
    own wrapped index list; output column i = 16*m + w holds row-w-of-group's
    m-th gathered value looked up in every partition of the group, so the
    valid value for partition p sits at column 16*m + (p % 16).
  Phase merge on the (otherwise idle) TensorEngine: 16 accumulated matmuls
    with diagonal 0/1 weights W_w = diag(p%16 == w) select the valid phase
    per partition into PSUM: packed[p,m] = sum_w W_w[p,p] * tmp[p, 16m+w].
    Exact in fp32 (one nonzero term per output).
  Range reduction with compare-wraps (is_gt/is_lt + mul-add) into
    [-pi, pi]; ACT Sin for sin, Sin(-|y|+pi/2) for cos; DVE products;
    per-class affine (ACT Copy with per-partition scale + DVE
    scalar_tensor_tensor); broadcast odd columns; DMA out.
"""

import os

os.environ.setdefault("BY_DEFAULT_DISABLE_SUBTILE_DEPS", "1")

import numpy as np

import concourse.bass as bass
import concourse.mybir as mybir
from concourse.tile import TileContext as _TileContext, ScopedClock

N_CORES = 8
B_FULL = 1024
B = B_FULL // N_CORES  # 128 rows per core
NPIX = 3072
NPAIR = 460
NIDX = 2 * NPAIR  # 920 gathered values per row
NCLS = 10
F32 = mybir.dt.float32
I32 = mybir.dt.int32
U16 = mybir.dt.uint16
ALU = mybir.AluOpType
PI = float(np.pi)
TWO_PI = float(2 * np.pi)
HALF_PI = float(np.pi / 2)


class TileContext(_TileContext):
    pass


def _legalize_sync_waits(nc):
    """This walrus build allows only ONE sync-wait per non-EventSemaphore
    instruction (and two on EventSemaphore). Tile's add_semaphores can attach
    several. Hoist excess waits onto EventSemaphore instructions inserted
    immediately before the owner on the same engine — semantically identical
    (same engine stream, waits run first)."""
    n_new = 0
    for f in nc.m.functions:
        for bb in f.blocks:
            out = []
            for inst in bb.instructions:
                si = inst.sync_info
                waits = list(si.on_wait) if si is not None and si.on_wait else []
                cap = 2 if inst.opcode == "EventSemaphore" else 1
                if len(waits) > cap:
                    keep, hoist = waits[:cap], waits[cap:]
                    del si.on_wait[:]
                    for w in keep:
                        si.on_wait.append(w)
                    while hoist:
                        chunk, hoist = hoist[:2], hoist[2:]
                        n_new += 1
                        ev = mybir.InstEventSemaphore(
                            name=f"{inst.name}-hw{n_new}",
                            ins=[],
                            outs=[],
                            engine=inst.engine,
                            sync_info=mybir.SyncInfo(on_wait=chunk, on_update=[]),
                        )
                        out.append(ev)
                out.append(inst)
            bb.instructions = out
    return nc


def build_kernel(n_chunks=4, n_repeat=1, pe_phases=12, span_chunks=(2, 1, 1), parts="gmtc"):
    """One NeuronCore's program: 128 batch rows.

    n_chunks: gather/merge/trig pipeline granularity (divides 920, even CH).
    pe_phases: how many of the 16 phase-merge terms run on the TensorEngine
      (accumulated in PSUM); the rest run as a masked mul-add chain on DVE.
    span_chunks: class/output-stage spans, in units of chunks (sums to
      n_chunks). A small final span shortens the non-overlapped tail.
    n_repeat: re-runs the whole pipeline (identical results) for timing.
    """
    Sin = mybir.ActivationFunctionType.Sin
    Copy = mybir.ActivationFunctionType.Copy
    Abs = mybir.ActivationFunctionType.Abs

    nc = bass.Bass(detect_race_conditions=False)
    xd = nc.dram_tensor("x", [B, NPIX], F32, kind="ExternalInput")
    pd = nc.dram_tensor("pidx", [B, NIDX], I32, kind="ExternalInput")
    td = nc.dram_tensor("theta", [1, NCLS], F32, kind="ExternalInput")
    od = nc.dram_tensor("out", [B, NIDX * NCLS], F32, kind="ExternalOutput")

    assert NIDX % n_chunks == 0
    CH = NIDX // n_chunks  # gathered values per chunk
    assert CH % 2 == 0
    PCH = CH // 2  # pairs per chunk
    assert sum(span_chunks) == n_chunks

    with TileContext(nc) as tc:
        with (
            tc.tile_pool(name="const", bufs=1) as cpool,
            tc.tile_pool(name="inp", bufs=1) as ipool,
            tc.tile_pool(name="tmp", bufs=3) as tpool,
            tc.tile_pool(name="mid", bufs=2) as mpool,
            tc.tile_pool(name="trig", bufs=1) as gpool,
            tc.tile_pool(name="outp", bufs=2) as opool,
            tc.tile_pool(name="tccp", bufs=4) as tccpool,
            tc.tile_pool(name="ps", bufs=2, space="PSUM") as ppool,
        ):
            # --- phase masks M[p, w] = 1.0 if p % 16 == w else 0.0 ---
            rowx = cpool.tile([B, 1], I32, tag="rowx")
            nc.gpsimd.iota(rowx[:], pattern=[[0, 1]], base=0, channel_multiplier=1)
            pm16 = cpool.tile([B, 1], I32, tag="pm16")
            nc.vector.tensor_scalar(pm16[:], rowx[:], 15, None, ALU.bitwise_and)
            wrow = cpool.tile([B, 16], I32, tag="wrow")
            nc.gpsimd.iota(wrow[:], pattern=[[1, 16]], base=0, channel_multiplier=0)
            M = cpool.tile([B, 16], F32, tag="M")
            nc.vector.tensor_tensor(
                M[:], pm16[:, 0:1].broadcast_to((B, 16)), wrow[:], ALU.is_equal
            )

            # --- coefficients: A = 0.5*cos(theta), Bc = -0.5*sin(theta) ---
            # (theta + indices ride the ACT HWDGE ring so the big x DMA on
            # the SP ring doesn't serialize ahead of them)
            th = cpool.tile([B, NCLS], F32, tag="th")
            nc.scalar.dma_start(out=th[:], in_=td[:].to_broadcast((B, NCLS)))
            halfpi = cpool.tile([B, 1], F32, tag="halfpi")
            nc.gpsimd.memset(halfpi[:], HALF_PI)
            zbias = cpool.tile([B, 1], F32, tag="zbias")
            nc.gpsimd.memset(zbias[:], 0.0)

            # Range reduction with standard ALUs (valid for |x| < 3pi):
            #   y = x - 2pi*(x > pi) + 2pi*(x < -pi)  in [-pi, pi]
            #   sin(x) = Sin(y);  cos(x) = cos(|y|) = Sin(-|y| + pi/2)
            def wrap2(dst_y, src, g, l, y1, cmp_engine):
                cmp_engine.tensor_scalar(g, src, PI, None, ALU.is_gt)
                cmp_engine.tensor_scalar(l, src, -PI, None, ALU.is_lt)
                nc.vector.scalar_tensor_tensor(
                    y1, g, -TWO_PI, src, ALU.mult, ALU.add
                )
                nc.vector.scalar_tensor_tensor(
                    dst_y, l, TWO_PI, y1, ALU.mult, ALU.add
                )

            thy = cpool.tile([B, NCLS], F32, tag="thy")
            thz = cpool.tile([B, NCLS], F32, tag="thz")
            tg = cpool.tile([B, NCLS], F32, tag="tg")
            tl = cpool.tile([B, NCLS], F32, tag="tl")
            t1 = cpool.tile([B, NCLS], F32, tag="t1")
            wrap2(thy[:], th[:], tg[:], tl[:], t1[:], nc.vector)
            nc.scalar.activation(thz[:], thy[:], Abs, bias=zbias[:, 0:1])
            A = cpool.tile([B, NCLS], F32, tag="A")
            Bc = cpool.tile([B, NCLS], F32, tag="Bc")
            nc.scalar.activation(A[:], thz[:], Sin, bias=halfpi[:, 0:1], scale=-1.0)
            nc.scalar.activation(Bc[:], thy[:], Sin, bias=zbias[:, 0:1])
            nc.vector.tensor_scalar_mul(A[:], A[:], 0.5)
            nc.vector.tensor_scalar_mul(Bc[:], Bc[:], -0.5)

            if pe_phases > 0:
                # --- diagonal selection weights W_w = diag(M[:, w]) ---
                colx = cpool.tile([B, 128], I32, tag="colx")
                nc.gpsimd.iota(colx[:], pattern=[[1, 128]], base=0, channel_multiplier=0)
                D = cpool.tile([B, 128], F32, tag="D")
                nc.vector.tensor_tensor(
                    D[:], rowx[:, 0:1].broadcast_to((B, 128)), colx[:], ALU.is_equal
                )
                W_all = cpool.tile([B, pe_phases * 128], F32, tag="W_all")
                for w in range(pe_phases):
                    nc.vector.tensor_scalar(
                        W_all[:, w * 128 : (w + 1) * 128],
                        D[:],
                        M[:, w : w + 1],
                        None,
                        ALU.mult,
                    )

            # --- inputs ---
            i32 = ipool.tile([B, NIDX], I32, tag="i32")
            nc.scalar.dma_start(out=i32[:], in_=pd[:])
            xt = ipool.tile([B, NPIX], F32, tag="xt")
            nc.sync.dma_start(out=xt[:], in_=xd[:])
            i16 = ipool.tile([B, NIDX], U16, tag="i16")
            nc.vector.tensor_copy(i16[:], i32[:].bitcast(U16)[:, 0 : 2 * NIDX : 2])

            if pe_phases > 0:
                # warm the PE (HAM) during the prologue so the first real
                # merge matmuls run at full rate
                pwarm = ppool.tile([B, 64], F32, tag="pwarm")
                for _ in range(6):
                    nc.tensor.matmul(
                        pwarm[:], W_all[:, 0:128], D[:, 0:64], start=True, stop=True
                    )

            def class_span(p0, PS, ca, sa, cb, sb):
                """Per-class output stage for pairs [p0, p0+PS)."""
                sl = slice(p0, p0 + PS)
                v = mpool.tile([B, PS], F32, tag="v")
                wv = mpool.tile([B, PS], F32, tag="wv")
                nc.vector.tensor_mul(v[:], sa[:, sl], sb[:, sl])
                nc.vector.tensor_mul(wv[:], ca[:, sl], cb[:, sl])

                ob = opool.tile([B, PS * 2 * NCLS], F32, tag="ob")
                ob3 = ob[:].rearrange("p (t k) -> p t k", k=2 * NCLS)

                om = mpool.tile([B, PS], F32, tag="om")
                nc.scalar.activation(om[:], wv[:], Copy, bias=0.5, scale=0.5)
                nc.scalar.activation(
                    ob3[:, :, NCLS : 2 * NCLS],
                    om[:, :, None].broadcast_to((B, PS, NCLS)),
                    Copy,
                )

                # even rows per class c: A_c*ca + (Bc_c*v + 0.5)
                for c in range(NCLS):
                    tcc = tccpool.tile([B, PS], F32, tag="tcc")
                    nc.scalar.activation(
                        tcc[:], v[:], Copy, bias=0.5, scale=Bc[:, c : c + 1]
                    )
                    nc.vector.scalar_tensor_tensor(
                        ob[:, c : PS * 2 * NCLS : 2 * NCLS],
                        ca[:, sl],
                        A[:, c : c + 1],
                        tcc[:],
                        ALU.mult,
                        ALU.add,
                    )

                nc.sync.dma_start(
                    out=od[:, p0 * 2 * NCLS : (p0 + PS) * 2 * NCLS],
                    in_=ob[:],
                )

            for rep in range(n_repeat):
                # full-width cos/sin buffers, filled per chunk
                ca = gpool.tile([B, NPAIR], F32, tag="ca")
                sa = gpool.tile([B, NPAIR], F32, tag="sa")
                cb = gpool.tile([B, NPAIR], F32, tag="cb")
                sb = gpool.tile([B, NPAIR], F32, tag="sb")

                for k in range(n_chunks):
                    # --- gather (this walrus caps indirect_copy at 64
                    # index-columns per instruction, so sub-gather) ---
                    tmp = tpool.tile([B, CH * 16], F32, tag="tmp")
                    GSUB = 64
                    for g0 in [] if "g" not in parts else range(0, CH, GSUB):
                        gn = min(GSUB, CH - g0)
                        nc.gpsimd.indirect_copy(
                            tmp[:, 16 * g0 : 16 * (g0 + gn)],
                            xt[:],
                            i16[:, k * CH + g0 : k * CH + g0 + gn],
                            True,
                        )

                    # --- phase merge -> pk[p, m] (interleaved a,b) ---
                    if "m" not in parts:
                        pk = mpool.tile([B, CH], F32, tag="acc")
                    elif pe_phases > 0:
                        pk = ppool.tile([B, CH], F32, tag="pk")
                        for w in range(pe_phases):
                            nc.tensor.matmul(
                                pk[:],
                                W_all[:, w * 128 : (w + 1) * 128],
                                tmp[:, w : CH * 16 : 16],
                                start=(w == 0),
                                stop=(w == pe_phases - 1),
                            )
                    if "m" in parts and pe_phases < 16:
                        w0 = pe_phases
                        acc = mpool.tile([B, CH], F32, tag="acc")
                        nc.vector.tensor_scalar(
                            acc[:], tmp[:, w0 : CH * 16 : 16], M[:, w0 : w0 + 1],
                            None, ALU.mult,
                        )
                        for w in range(w0 + 1, 16):
                            nc.vector.scalar_tensor_tensor(
                                acc[:],
                                tmp[:, w : CH * 16 : 16],
                                M[:, w : w + 1],
                                acc[:],
                                ALU.mult,
                                ALU.add,
                            )
                        if pe_phases > 0:
                            nc.vector.tensor_add(acc[:], acc[:], pk[:])
                        pk = acc

                    # --- range reduction + trig -> full-width buffers ---
                    # (pk is SBUF when the DVE-assist merge ran; evacuate
                    # PSUM via ACT first when the full merge was on PE, so
                    # the GPSIMD compares can read it)
                    if pe_phases == 16:
                        pksb = mpool.tile([B, CH], F32, tag="pksb")
                        nc.scalar.activation(pksb[:], pk[:], Copy)
                        pk = pksb
                    av = pk[:, 0:CH:2]
                    bv = pk[:, 1:CH:2]
                    aw = mpool.tile([B, PCH], F32, tag="aw")
                    ac = mpool.tile([B, PCH], F32, tag="ac")
                    bw = mpool.tile([B, PCH], F32, tag="bw")
                    bc2 = mpool.tile([B, PCH], F32, tag="bc2")
                    ga = mpool.tile([B, PCH], F32, tag="ga")
                    la = mpool.tile([B, PCH], F32, tag="la")
                    gb = mpool.tile([B, PCH], F32, tag="gb")
                    lb = mpool.tile([B, PCH], F32, tag="lb")
                    if "t" not in parts:
                        continue
                    wrap2(aw[:], av, ga[:], la[:], ac[:], nc.vector)
                    nc.scalar.activation(ac[:], aw[:], Abs, bias=zbias[:, 0:1])
                    wrap2(bw[:], bv, gb[:], lb[:], bc2[:], nc.vector)
                    nc.scalar.activation(bc2[:], bw[:], Abs, bias=zbias[:, 0:1])

                    sl = slice(k * PCH, (k + 1) * PCH)
                    nc.scalar.activation(
                        ca[:, sl], ac[:], Sin, bias=halfpi[:, 0:1], scale=-1.0
                    )
                    nc.scalar.activation(sa[:, sl], aw[:], Sin, bias=zbias[:, 0:1])
                    nc.scalar.activation(
                        cb[:, sl], bc2[:], Sin, bias=halfpi[:, 0:1], scale=-1.0
                    )
                    nc.scalar.activation(sb[:, sl], bw[:], Sin, bias=zbias[:, 0:1])

                    # interleave the class/output stage as soon as its
                    # span of pairs is complete
                    done = k + 1
                    acc_ch = 0
                    for nch in [] if "c" not in parts else span_chunks:
                        if acc_ch + nch == done:
                            class_span(acc_ch * PCH, nch * PCH, ca, sa, cb, sb)
                            break
                        acc_ch += nch
    return _legalize_sync_waits(nc)


def _prep_inputs(x, theta, pair_idx):
    """Full inputs -> list of per-core input maps (host-side sharding only)."""
    x = np.ascontiguousarray(np.asarray(x, dtype=np.float32).reshape(B_FULL, NPIX))
    theta = np.ascontiguousarray(np.asarray(theta, dtype=np.float32).reshape(1, NCLS))
    pidx = np.asarray(pair_idx)
    assert pidx.shape == (B_FULL, NPAIR, 2), pidx.shape
    if pidx.dtype != np.int32:
        pidx = pidx.astype(np.int32)  # value-preserving narrowing for the DMA
    pidx = np.ascontiguousarray(pidx.reshape(B_FULL, NIDX))
    in_maps = []
    for k in range(N_CORES):
        sl = slice(k * B, (k + 1) * B)
        in_maps.append({"x": x[sl], "pidx": pidx[sl], "theta": theta})
    return in_maps


_CACHED = {}


def kernel(x, theta, pair_idx):
    from concourse.bass_utils import run_bass_kernel_spmd

    if "nc" not in _CACHED:
        _CACHED["nc"] = build_kernel()
    nc = _CACHED["nc"]
    in_maps = _prep_inputs(x, theta, pair_idx)
    res = run_bass_kernel_spmd(nc, in_maps, core_ids=list(range(N_CORES)))
    out = np.concatenate([r["out"] for r in res.results], axis=0)
    return out.reshape(B_FULL, NIDX, NCLS)
